# revision 15
# baseline (speedup 1.0000x reference)
"""EuclideanPairwiseDistances kernel for 8 TRN2 NeuronCores.

Problem: input [B=4, H=256, L=1024, N=128] f32, mask [B, L, N] bool.
  y[b,h,n] = masked mean of input over l=1..1023  -> [B, H, N]
  out[b,p] = sqrt(sum_h (y[b,:,i_p] - y[b,:,j_p])^2 + eps) over tril pairs.

Sharding: core c handles batch b=c//2 and H-half h0=128*(c%2).  The host
casts each core's x-slice to f16 (rel tol is 2e-2; f16 rounding costs
~5e-4), HALVING the HBM traffic - the kernel is DMA-fabric-bound
(16 engines x ~25 B/ns = ~410 GB/s/core).  Planes are stored pair-
interleaved on the host so each SBUF partition's contiguous DRAM run
stays 4 KiB (full DMA packet efficiency at f16).

On chip: z = x16*md16 on the DVE (f16 2x perf mode, one fused multiply
per DMA half so it runs ~10% under the DMA streaming rate), masked sums
S[n,h] via PE matmuls with a ones vector, then partial squared pairwise
distances over the core's 128 h-dims via a Gram-matrix trick.  Host adds
the two halves per batch, applies sqrt, and extracts the tril pairs.

The mask, the 1/denom division, the CLS (l=0) exclusion and a 2^10 scale
(keeps f16 intermediates in range) are folded into one host-side f16
tensor md[l,n].
"""

import numpy as np

import concourse.mybir as mybir
import concourse.tile as tile
from concourse import bacc
from concourse.bass_utils import run_bass_kernel_spmd
from concourse.masks import make_identity

B, H, L, N = 4, 256, 1024, 128
HSH = 128          # h-dims per core
PL = 8             # l-values per partition (L = 128 * PL)
NU = HSH // 2      # pair-units: two h-planes interleaved per 4 KiB run
UG = 4             # pair-units per big DMA group (8 planes, 2 MiB)
EPS = 1e-8
C = 1024.0         # scale folded into md; keeps z=x*md*C/denom ~ O(1) in f16

X_BUFS = 14        # half-group (1 MiB) tiles
Z_BUFS = 6
FRESH_HALVES = 4   # last half-groups get dedicated (never-recycled) x tiles
                   # so their DMAs carry no buffer-reuse semaphore waits

_cached = {}


def _build_bass():
    nc = bacc.Bacc("TRN2", target_bir_lowering=False)

    # xs: pair-interleaved f16 layout [pair, l/8, 2, 8, n] so partition p's
    # source run (2 planes x 8 l x 128 n x 2B) is 4 KiB contiguous
    xs = nc.dram_tensor(
        "xs", [NU, 128, 2, PL, N], mybir.dt.float16, kind="ExternalInput"
    )
    md = nc.dram_tensor("md", [L, N], mybir.dt.float16, kind="ExternalInput")
    dout = nc.dram_tensor("dout", [N, N], mybir.dt.float32, kind="ExternalOutput")

    f16 = mybir.dt.float16
    f32 = mybir.dt.float32

    # groups in pair-units: two half-size groups at the head (fast queue
    # fill + early DVE start), then uniform big groups to the end
    groups = [(0, 2), (2, 2)]
    u = 4
    while u < NU:
        groups.append((u, UG))
        u += UG

    with tile.TileContext(nc) as tc:
        with (
            tc.tile_pool(name="xp", bufs=X_BUFS) as xp,
            tc.tile_pool(name="zp", bufs=Z_BUFS) as zp,
            tc.tile_pool(name="singles", bufs=1) as singles,
            tc.tile_pool(name="st2", bufs=1) as st2,
            tc.tile_pool(name="psum", bufs=1, space="PSUM") as psum,
        ):
            # --- one-time setup: md split-loaded on both HWDGE rings so it
            # lands before the first x group (it gates the first multiply) ---
            md_t = singles.tile([128, PL, N], f16)
            md_src = md.rearrange("(p s) n -> p s n", p=128)
            nc.sync.dma_start(out=md_t[:, : PL // 2], in_=md_src[:, : PL // 2])
            nc.scalar.dma_start(out=md_t[:, PL // 2 :], in_=md_src[:, PL // 2 :])

            ones_col = singles.tile([128, 1], f16)
            nc.vector.memset(ones_col, 1.0)
            ones_mat = singles.tile([128, 128], f16)
            nc.vector.memset(ones_mat, 1.0)
            ident = singles.tile([128, 128], f16)
            make_identity(nc, ident)

            # --- stage 1: masked sums S[n, h] (C-scaled) ---
            s_psum = psum.tile([N, HSH], f32)
            d_psum = psum.tile([N, N], f32)

            # stage 2, one h-half at a time: PSUM columns [hlo, hhi) are fully
            # accumulated once those planes' matmul groups retire, so the first
            # half's transpose/Gram work hides under the second half's stream.
            def stage2_half(hi):
                hlo, hhi = hi * (HSH // 2), (hi + 1) * (HSH // 2)
                hw = hhi - hlo
                y_nh = st2.tile([N, HSH // 2], f16, tag=f"y{hi}")
                nc.vector.tensor_copy(y_nh, s_psum[:, hlo:hhi])
                yt_ps = psum.tile([HSH // 2, N], f16, tag=f"ytp{hi}")
                nc.tensor.transpose(yt_ps, y_nh, ident)
                yt = st2.tile([HSH // 2, N], f16, tag=f"yt{hi}")
                nc.vector.tensor_copy(yt, yt_ps)
                ym2 = st2.tile([HSH // 2, N], f16, tag=f"ym{hi}")
                nc.vector.tensor_scalar_mul(ym2, yt_ps, -2.0)
                ysq = st2.tile([HSH // 2, N], f16, tag=f"ys{hi}")
                nc.vector.tensor_mul(ysq, yt, yt)
                first, last = (hi == 0), (hi == 1)
                nc.tensor.matmul(d_psum, yt, ym2, start=first, stop=False)
                nc.tensor.matmul(
                    d_psum, ones_mat[:hw], ysq, start=False, stop=False
                )
                nc.tensor.matmul(
                    d_psum, ysq, ones_mat[:hw], start=False, stop=last
                )

            for gi, (u0, gsz) in enumerate(groups):
                # per-half tiles + fused DVE multiply per half (md broadcast
                # across the plane axes, f16 2x perf mode): fine buffer
                # granularity keeps tail DMAs off recent WAR semaphores
                hf = gsz // 2
                for half in range(2):
                    ub = u0 + half * hf
                    fresh = ub + hf > NU - FRESH_HALVES * (UG // 2)
                    if fresh:
                        x_t = singles.tile(
                            [128, UG // 2, 2, PL, N], f16, tag=f"xf{ub}"
                        )
                    else:
                        x_t = xp.tile([128, UG // 2, 2, PL, N], f16, tag="x")
                    src = xs[ub : ub + hf].rearrange("k p t s n -> p k t s n")
                    z_t = zp.tile([128, UG // 2, 2, PL, N], f16, tag="z")
                    eng = nc.sync if half == 0 else nc.scalar
                    eng.dma_start(out=x_t[:, :hf], in_=src)
                    md_b = md_t[:, None, None].broadcast_to([128, hf, 2, PL, N])
                    nc.vector.tensor_mul(z_t[:, :hf], x_t[:, :hf], md_b)

                    for uu in range(hf):
                        for t in range(2):
                            h = (ub + uu) * 2 + t
                            for ls in range(PL):
                                nc.tensor.matmul(
                                    s_psum[:, h : h + 1],
                                    z_t[:, uu, t, ls, :],
                                    ones_col,
                                    start=(ls == 0),
                                    stop=(ls == PL - 1),
                                )
                if (u0 + gsz) * 2 == HSH // 2:
                    stage2_half(0)

            stage2_half(1)
            d_sb = st2.tile([N, N], f32)
            nc.vector.tensor_copy(d_sb, d_psum)
            nc.sync.dma_start(out=dout[:, :], in_=d_sb)

    nc.compile()
    return nc


def get_bass():
    if "nc" not in _cached:
        _cached["nc"] = _build_bass()
    return _cached["nc"]


def _host_prep(input, mask):
    """Returns per-core in_maps."""
    input = np.asarray(input, dtype=np.float32)
    mask = np.asarray(mask)
    denom = mask[:, 1:, :].sum(axis=1)                    # [B, N] ints
    denom = np.maximum(denom, 1).astype(np.float32)
    md = mask.astype(np.float32) * (np.float32(C) / denom[:, None, :])
    md[:, 0, :] = 0.0                                     # CLS position excluded
    md = np.ascontiguousarray(md.astype(np.float16))

    # pair-interleaved f16 x: [B, H] -> per core [NU, 2, L/8, 8, N]
    # -> stored as [NU, L/8, 2, 8, N] so partition runs are 4 KiB
    x16 = input.astype(np.float16).reshape(B, H, 128, PL, N)
    in_maps = []
    for c in range(8):
        b, half = c // 2, c % 2
        xc = x16[b, half * HSH : (half + 1) * HSH]        # [HSH, 128, 8, N]
        xc = xc.reshape(NU, 2, 128, PL, N).transpose(0, 2, 1, 3, 4)
        in_maps.append(
            {
                "xs": np.ascontiguousarray(xc),
                "md": md[b],
            }
        )
    return in_maps


def _host_post(results):
    d = np.stack([r["dout"] for r in results])            # [8, 128, 128]
    dsum = (d[0::2].astype(np.float64) + d[1::2].astype(np.float64)) / (C * C)
    dist = np.sqrt(np.maximum(dsum, 0.0) + EPS).astype(np.float32)  # [4, 128, 128]
    i, j = np.tril_indices(N, -1)
    return np.ascontiguousarray(dist[:, i, j])


def kernel(input, mask, _run_kwargs=None):
    nc = get_bass()
    in_maps = _host_prep(input, mask)
    kwargs = _run_kwargs or {}
    res = run_bass_kernel_spmd(nc, in_maps, core_ids=list(range(8)), **kwargs)
    out = _host_post(res.results)
    if kwargs:
        _cached["last_result"] = res
    return out
